# revision 14
# baseline (speedup 1.0000x reference)
"""Trainium2 Bass kernel for the BaseEnergyFormPhysics tet-mesh potential energy.

Strategy (per sharding hint): partition the 2M elements across the 8
NeuronCores.  The host shards conns, gathers the per-element nodal data
(coords/us -> element edge vectors / displacement diffs), and packs it into
dense per-core component planes.  Each core streams its element planes from
HBM and computes, fully on-device, the cross products r_n, det(J), the
displacement gradient G = sum_n w_n (x) r_n, the strain invariants
S = sum(eps^2), T = tr(eps) (scaled by det), and three partial sums

    A = sum S * 1/|det|
    B = sum T^2 * 1/|det|
    C = sum (sum_n u_nz) * |det|

per partition lane.  The scalar energy is unsharded on the host:
    E = mu/6 * A + lam/12 * B - rho/24 * C  (summed over cores/lanes/chunks)

Precision: the det path (edges, cross products r1/r2, det, 1/|det|) runs in
fp32; the strain path (w, G, S, T) runs in fp16, which costs nothing in
accuracy (the error budget is dominated by fp32 near-singular tets) and
doubles DVE throughput.  Degenerate elements (duplicate node -> det exactly
0) produce 1/0 -> NaN on device, matching the NaN the jax reference yields
for this input.
"""

import numpy as np
from contextlib import ExitStack

import concourse.bass as bass
import concourse.bacc as bacc
import concourse.tile as tile
import concourse.mybir as mybir
from concourse.bass_utils import run_bass_kernel_spmd

F32 = mybir.dt.float32
F16 = mybir.dt.float16
AX = mybir.AxisListType
ALU = mybir.AluOpType
ACTF = mybir.ActivationFunctionType

N_CORES = 8
P = 128
N_ELEMS = 2_000_000
E_CORE = N_ELEMS // N_CORES          # 250_000 elements per core
CHUNK_F = 489                        # free-dim columns per chunk
N_CHUNKS = 4
COLS = CHUNK_F * N_CHUNKS            # 1956 columns  (128*1956 = 250368 >= 250000)
E_PAD = P * COLS

# fp32 planes: e1(3) e2(3) e3(3) uqzsum(1)
# fp16 planes: A(6) B(6) C(6) D(6) = pre-rotated e arrangements for the packed
# r2/r3 cross products (r2 term1 = A[0:3]*B[0:3], r3 term1 = A[3:6]*B[3:6],
# term2s from C*D), then w1..w3 (9)
N_P32 = 10
N_P16 = 33

_CACHE = {}


def _build(chunk_f=CHUNK_F, n_chunks=N_CHUNKS):
    cols = chunk_f * n_chunks
    nc = bacc.Bacc("TRN2", target_bir_lowering=False, debug=False,
                   num_devices=N_CORES)
    pl32 = nc.dram_tensor("planes32", [P, N_P32, cols], F32,
                          kind="ExternalInput").ap()
    pl16 = nc.dram_tensor("planes16", [P, N_P16, cols], F16,
                          kind="ExternalInput").ap()
    out = nc.dram_tensor("out", [P, 3, n_chunks], F32,
                         kind="ExternalOutput").ap()

    with tile.TileContext(nc) as tc, ExitStack() as ctx:
        in_pool = ctx.enter_context(tc.tile_pool(name="inp", bufs=2))
        r_pool = ctx.enter_context(tc.tile_pool(name="rp", bufs=2))
        g_pool = ctx.enter_context(tc.tile_pool(name="gp", bufs=1))
        s_pool = ctx.enter_context(tc.tile_pool(name="sp", bufs=1))
        a_pool = ctx.enter_context(tc.tile_pool(name="acp", bufs=1))

        acc = a_pool.tile([P, 3, n_chunks], F32)

        for c in range(n_chunks):
            F = chunk_f
            # split DMAs so the planes feeding the critical engine (DVE
            # cross products read i16[9:18]) land first
            i16 = in_pool.tile([P, N_P16, F], F16, tag="i16")
            nc.sync.dma_start(i16[:, 0:24, :], pl16[:, 0:24, c * F:(c + 1) * F])
            i32 = in_pool.tile([P, N_P32, F], F32, tag="i32")
            nc.sync.dma_start(i32[:, 0:9, :], pl32[:, 0:9, c * F:(c + 1) * F])
            nc.sync.dma_start(i16[:, 24:33, :], pl16[:, 24:33, c * F:(c + 1) * F])
            nc.sync.dma_start(i32[:, 9:10, :], pl32[:, 9:10, c * F:(c + 1) * F])

            e_blk = i32[:, 0:9, :]       # fp32 (n, comp): e1x..e3z
            uqz = i32[:, 9, :]           # [P, F] fp32
            w_blk = i16[:, 24:33, :]     # fp16 (n, comp): w1x..w3z

            # --- cross products: r2 = e3 x e1, r3 = e1 x e2 in fp16 (DVE,
            #     3 packed ops over the host-pre-rotated A/B/C/D blocks);
            #     r1 = e2 x e3 in fp32 (GPSIMD, feeds det)
            r = r_pool.tile([P, 3, F], F32, tag="r32")      # r1 only
            rh = r_pool.tile([P, 9, F], F16, tag="rh")      # fp16 r1,r2,r3
            G = g_pool.tile([P, 3, 3, F], F16)
            gt = g_pool.tile([P, 3, 3, F], F16)
            gtf = gt[:].rearrange("p i d f -> p (i d) f")
            vtmp = gtf[:, 0:6, :]
            nc.vector.tensor_mul(rh[:, 3:9, :], i16[:, 0:6, :], i16[:, 6:12, :])
            nc.vector.tensor_mul(vtmp, i16[:, 12:18, :], i16[:, 18:24, :])
            nc.vector.tensor_sub(rh[:, 3:9, :], rh[:, 3:9, :], vtmp)
            # fp16 r1 for the G path, also on DVE (decouples G from the
            # GPSIMD r1-fp32 -> det chain): r1 = rot1(e2)*rot2(e3) -
            # rot2(e2)*rot1(e3), all blocks already shipped
            vtmp2 = gtf[:, 6:9, :]
            nc.vector.tensor_mul(rh[:, 0:3, :], i16[:, 21:24, :], i16[:, 12:15, :])
            nc.vector.tensor_mul(vtmp2, i16[:, 9:12, :], i16[:, 0:3, :])
            nc.vector.tensor_sub(rh[:, 0:3, :], rh[:, 0:3, :], vtmp2)
            for k, (i, j) in enumerate(((1, 2), (2, 0), (0, 1))):
                t1 = r_pool.tile([P, F], F32, tag="crt1")
                nc.gpsimd.tensor_mul(t1[:], e_blk[:, 3 + i, :], e_blk[:, 6 + j, :])
                t2 = r_pool.tile([P, F], F32, tag="crt2")
                nc.gpsimd.tensor_mul(t2[:], e_blk[:, 3 + j, :], e_blk[:, 6 + i, :])
                nc.gpsimd.tensor_sub(r[:, k, :], t1[:], t2[:])

            # --- det = e1 . r1   (fp32, GPSIMD)
            dp = s_pool.tile([P, 3, F], F32, tag="dp")
            nc.gpsimd.tensor_mul(dp[:], e_blk[:, 0:3, :], r[:])
            dta = s_pool.tile([P, F], F32, tag="dta")
            nc.gpsimd.tensor_add(dta[:], dp[:, 0, :], dp[:, 1, :])
            det = s_pool.tile([P, F], F32, tag="det")
            nc.gpsimd.tensor_add(det[:], dta[:], dp[:, 2, :])
            adet = s_pool.tile([P, F], F32, tag="adet")
            nc.scalar.activation(adet[:], det[:], ACTF.Abs)
            # 1/|det|; exact-zero |det| (duplicate-node elements) -> NaN,
            # which must propagate into A and B to match the reference.
            recip = s_pool.tile([P, F], F32, tag="recip")
            nc.vector.reciprocal_approx_fast(out=recip[:], in_=adet[:])

            # --- G[i,d] = sum_n w_n[i] * r_n[d]   (fp16, broadcast APs)
            # accumulate the r1 term last: it has the longest dependency
            # chain (GPSIMD crosses -> ScalarE downconvert -> DVE)
            w2 = w_blk[:, 3:6, :].unsqueeze(2).broadcast_to([P, 3, 3, F])
            r2b = rh[:, 3:6, :].unsqueeze(1).broadcast_to([P, 3, 3, F])
            nc.vector.tensor_mul(G[:], w2, r2b)
            w3 = w_blk[:, 6:9, :].unsqueeze(2).broadcast_to([P, 3, 3, F])
            r3b = rh[:, 6:9, :].unsqueeze(1).broadcast_to([P, 3, 3, F])
            nc.vector.tensor_mul(gt[:], w3, r3b)
            nc.vector.tensor_add(G[:], G[:], gt[:])
            w1 = w_blk[:, 0:3, :].unsqueeze(2).broadcast_to([P, 3, 3, F])
            r1b = rh[:, 0:3, :].unsqueeze(1).broadcast_to([P, 3, 3, F])
            nc.vector.tensor_mul(gt[:], w1, r1b)
            nc.vector.tensor_add(G[:], G[:], gt[:])

            # --- S = sum_i G_ii^2 + 0.5 * sum_{i<j} (G_ij + G_ji)^2
            sqb = s_pool.tile([P, 6, F], F16, tag="sqb")
            for i in range(3):
                nc.scalar.activation(sqb[:, i, :], G[:, i, i, :], ACTF.Square)
            for k, (i, j) in enumerate(((0, 1), (0, 2), (1, 2))):
                nc.vector.tensor_add(sqb[:, 3 + k, :], G[:, i, j, :], G[:, j, i, :])
            nc.scalar.activation(sqb[:, 3:6, :], sqb[:, 3:6, :], ACTF.Square,
                                 scale=0.7071067811865476)
            sa = s_pool.tile([P, 3, F], F16, tag="sa")
            nc.vector.tensor_add(sa[:], sqb[:, 0:3, :], sqb[:, 3:6, :])
            s1 = s_pool.tile([P, F], F16, tag="s1")
            nc.vector.tensor_add(s1[:], sa[:, 0, :], sa[:, 1, :])
            S = s_pool.tile([P, F], F32, tag="S")
            nc.vector.tensor_add(S[:], s1[:], sa[:, 2, :])

            # --- T = tr(G);  T2 = T^2 (fp32 out)
            tt = s_pool.tile([P, F], F16, tag="tt")
            nc.vector.tensor_add(tt[:], G[:, 0, 0, :], G[:, 1, 1, :])
            T = s_pool.tile([P, F], F16, tag="T")
            nc.vector.tensor_add(T[:], tt[:], G[:, 2, 2, :])
            T2 = s_pool.tile([P, F], F32, tag="T2")
            nc.scalar.activation(T2[:], T[:], ACTF.Square)

            # --- partial sums (free-dim reduction on ScalarE via accum_out)
            junk = s_pool.tile([P, F], F16, tag="junk")
            scr = s_pool.tile([P, F], F32, tag="scr")
            nc.vector.tensor_mul(scr[:], S[:], recip[:])
            nc.scalar.activation(junk[:], scr[:], ACTF.Copy,
                                 accum_out=acc[:, 0, c:c + 1])
            scr2 = s_pool.tile([P, F], F32, tag="scr2")
            nc.vector.tensor_mul(scr2[:], T2[:], recip[:])
            nc.scalar.activation(junk[:], scr2[:], ACTF.Copy,
                                 accum_out=acc[:, 1, c:c + 1])
            scr3 = s_pool.tile([P, F], F32, tag="scr3")
            nc.vector.tensor_mul(scr3[:], uqz, adet[:])
            nc.scalar.activation(junk[:], scr3[:], ACTF.Copy,
                                 accum_out=acc[:, 2, c:c + 1])

        nc.sync.dma_start(out[:], acc[:])

    nc.compile()
    return nc


def _get_nc():
    if "nc" not in _CACHE:
        _CACHE["nc"] = _build()
    return _CACHE["nc"]


def _pack_core(conns_i, coords, us):
    """Gather + pack one core's elements into the two plane tensors."""
    xs = coords[conns_i]                     # [E,4,3]
    ue = us[conns_i]                         # [E,4,3]
    e = xs[:, 1:4, :] - xs[:, 0:1, :]        # [E,3,3]  (n, comp)
    w = ue[:, 1:4, :] - ue[:, 0:1, :]        # [E,3,3]
    uqzsum = ue[:, :, 2].sum(axis=1)         # [E]
    E = conns_i.shape[0]
    pad_e = np.array([1, 0, 0, 0, 1, 0, 0, 0, 1], dtype=np.float32)

    p32 = np.empty((N_P32, E_PAD), dtype=np.float32)
    p32[0:9, :E] = e.reshape(E, 9).T
    p32[9, :E] = uqzsum
    p32[0:9, E:] = pad_e[:, None]
    p32[9, E:] = 0.0

    p16 = np.empty((N_P16, E_PAD), dtype=np.float16)
    eh = np.empty((3, 3, E_PAD), dtype=np.float16)          # [vec n, comp, elem]
    eh[:, :, :E] = e.reshape(E, 9).T.astype(np.float16).reshape(3, 3, E)
    eh[:, :, E:] = pad_e.astype(np.float16).reshape(3, 3)[:, :, None]
    rot1 = eh[:, [1, 2, 0], :]                              # (vy, vz, vx)
    rot2 = eh[:, [2, 0, 1], :]                              # (vz, vx, vy)
    # A = [rot1(e3), rot1(e1)]; B = [rot2(e1), rot2(e2)]
    # C = [rot2(e3), rot2(e1)]; D = [rot1(e1), rot1(e2)]
    p16[0:3] = rot1[2]; p16[3:6] = rot1[0]
    p16[6:9] = rot2[0]; p16[9:12] = rot2[1]
    p16[12:15] = rot2[2]; p16[15:18] = rot2[0]
    p16[18:21] = rot1[0]; p16[21:24] = rot1[1]
    p16[24:33, :E] = w.reshape(E, 9).T.astype(np.float16)
    p16[24:33, E:] = 0.0

    return (np.ascontiguousarray(p32.reshape(N_P32, P, COLS).transpose(1, 0, 2)),
            np.ascontiguousarray(p16.reshape(N_P16, P, COLS).transpose(1, 0, 2)))


def kernel(params, coords, us, t, conns):
    params = np.asarray(params, np.float32)
    coords = np.asarray(coords, np.float32)
    us = np.asarray(us, np.float32)
    conns = np.asarray(conns)
    lam, mu, rho = (np.float64(params[0]), np.float64(params[1]),
                    np.float64(params[2]))

    nc = _get_nc()
    in_maps = []
    for i in range(N_CORES):
        ci = conns[i * E_CORE:(i + 1) * E_CORE]
        a32, a16 = _pack_core(ci, coords, us)
        in_maps.append({"planes32": a32, "planes16": a16})

    res = run_bass_kernel_spmd(nc, in_maps, core_ids=list(range(N_CORES)),
                               trace=bool(_CACHE.get("trace", False)))
    _CACHE["last_results"] = res

    total = np.float64(0.0)
    for i in range(N_CORES):
        o = res.results[i]["out"].astype(np.float64)   # [P, 3, N_CHUNKS]
        A = o[:, 0, :].sum()
        B = o[:, 1, :].sum()
        C = o[:, 2, :].sum()
        total += mu / 6.0 * A + lam / 12.0 * B - rho / 24.0 * C
    return np.asarray(total, dtype=np.float32)


# revision 15
# speedup vs baseline: 1.0881x; 1.0881x over previous
"""Trainium2 Bass kernel for the BaseEnergyFormPhysics tet-mesh potential energy.

Strategy (per sharding hint): partition the 2M elements across the 8
NeuronCores.  The host shards conns, gathers the per-element nodal data
(coords/us -> element edge vectors / displacement diffs), and packs it into
dense per-core component planes.  Each core streams its element planes from
HBM and computes, fully on-device, the cross products r_n, det(J), the
displacement gradient G = sum_n w_n (x) r_n, the strain invariants
S = sum(eps^2), T = tr(eps) (scaled by det), and three partial sums

    A = sum S * 1/|det|
    B = sum T^2 * 1/|det|
    C = sum (sum_n u_nz) * |det|

per partition lane.  The scalar energy is unsharded on the host:
    E = mu/6 * A + lam/12 * B - rho/24 * C  (summed over cores/lanes/chunks)

Precision: the det path (edges, cross products r1/r2, det, 1/|det|) runs in
fp32; the strain path (w, G, S, T) runs in fp16, which costs nothing in
accuracy (the error budget is dominated by fp32 near-singular tets) and
doubles DVE throughput.  Degenerate elements (duplicate node -> det exactly
0) produce 1/0 -> NaN on device, matching the NaN the jax reference yields
for this input.
"""

import numpy as np
from contextlib import ExitStack

import concourse.bass as bass
import concourse.bacc as bacc
import concourse.tile as tile
import concourse.mybir as mybir
from concourse.bass_utils import run_bass_kernel_spmd

F32 = mybir.dt.float32
F16 = mybir.dt.float16
AX = mybir.AxisListType
ALU = mybir.AluOpType
ACTF = mybir.ActivationFunctionType

N_CORES = 8
P = 128
N_ELEMS = 2_000_000
E_CORE = N_ELEMS // N_CORES          # 250_000 elements per core
CHUNK_F = 489                        # free-dim columns per chunk
N_CHUNKS = 4
COLS = CHUNK_F * N_CHUNKS            # 1956 columns  (128*1956 = 250368 >= 250000)
E_PAD = P * COLS

# fp32 planes: e1(3) e2(3) e3(3) uqzsum(1)
# fp16 planes: A(6) B(6) C(6) D(6) = pre-rotated e arrangements for the packed
# r2/r3 cross products (r2 term1 = A[0:3]*B[0:3], r3 term1 = A[3:6]*B[3:6],
# term2s from C*D), then w1..w3 (9)
N_P32 = 10
N_P16 = 33

_CACHE = {}


def _build(chunk_f=CHUNK_F, n_chunks=N_CHUNKS):
    cols = chunk_f * n_chunks
    nc = bacc.Bacc("TRN2", target_bir_lowering=False, debug=False,
                   num_devices=N_CORES)
    pl32 = nc.dram_tensor("planes32", [P, N_P32, cols], F32,
                          kind="ExternalInput").ap()
    pl16 = nc.dram_tensor("planes16", [P, N_P16, cols], F16,
                          kind="ExternalInput").ap()
    out = nc.dram_tensor("out", [P, 3, n_chunks], F32,
                         kind="ExternalOutput").ap()

    with tile.TileContext(nc) as tc, ExitStack() as ctx:
        in_pool = ctx.enter_context(tc.tile_pool(name="inp", bufs=2))
        r_pool = ctx.enter_context(tc.tile_pool(name="rp", bufs=2))
        g_pool = ctx.enter_context(tc.tile_pool(name="gp", bufs=1))
        s_pool = ctx.enter_context(tc.tile_pool(name="sp", bufs=1))
        a_pool = ctx.enter_context(tc.tile_pool(name="acp", bufs=1))

        acc = a_pool.tile([P, 3, n_chunks], F32)

        for c in range(n_chunks):
            F = chunk_f
            # split DMAs so the planes feeding the critical engine (DVE
            # cross products read i16[9:18]) land first
            i16 = in_pool.tile([P, N_P16, F], F16, tag="i16")
            nc.sync.dma_start(i16[:, 0:24, :], pl16[:, 0:24, c * F:(c + 1) * F])
            i32 = in_pool.tile([P, N_P32, F], F32, tag="i32")
            nc.sync.dma_start(i32[:, 0:9, :], pl32[:, 0:9, c * F:(c + 1) * F])
            nc.sync.dma_start(i16[:, 24:33, :], pl16[:, 24:33, c * F:(c + 1) * F])
            nc.sync.dma_start(i32[:, 9:10, :], pl32[:, 9:10, c * F:(c + 1) * F])

            e_blk = i32[:, 0:9, :]       # fp32 (n, comp): e1x..e3z
            uqz = i32[:, 9, :]           # [P, F] fp32
            w_blk = i16[:, 24:33, :]     # fp16 (n, comp): w1x..w3z

            # --- cross products: r2 = e3 x e1, r3 = e1 x e2 in fp16 (DVE,
            #     3 packed ops over the host-pre-rotated A/B/C/D blocks);
            #     r1 = e2 x e3 in fp32 (GPSIMD, feeds det)
            r = r_pool.tile([P, 3, F], F32, tag="r32")      # r1 only
            rh = r_pool.tile([P, 9, F], F16, tag="rh")      # fp16 r1,r2,r3
            G = g_pool.tile([P, 3, 3, F], F16)
            gt = g_pool.tile([P, 3, 3, F], F16)
            vtmp = gt[:].rearrange("p i d f -> p (i d) f")[:, 0:6, :]
            nc.vector.tensor_mul(rh[:, 3:9, :], i16[:, 0:6, :], i16[:, 6:12, :])
            nc.vector.tensor_mul(vtmp, i16[:, 12:18, :], i16[:, 18:24, :])
            nc.vector.tensor_sub(rh[:, 3:9, :], rh[:, 3:9, :], vtmp)
            for k, (i, j) in enumerate(((1, 2), (2, 0), (0, 1))):
                t1 = r_pool.tile([P, F], F32, tag="crt1")
                nc.gpsimd.tensor_mul(t1[:], e_blk[:, 3 + i, :], e_blk[:, 6 + j, :])
                t2 = r_pool.tile([P, F], F32, tag="crt2")
                nc.gpsimd.tensor_mul(t2[:], e_blk[:, 3 + j, :], e_blk[:, 6 + i, :])
                nc.gpsimd.tensor_sub(r[:, k, :], t1[:], t2[:])
            # fp16 r1 for the G path (ScalarE does the downconvert copy)
            nc.scalar.activation(rh[:, 0:3, :], r[:], ACTF.Copy)

            # --- det = e1 . r1   (fp32, GPSIMD)
            dp = s_pool.tile([P, 3, F], F32, tag="dp")
            nc.gpsimd.tensor_mul(dp[:], e_blk[:, 0:3, :], r[:])
            dta = s_pool.tile([P, F], F32, tag="dta")
            nc.gpsimd.tensor_add(dta[:], dp[:, 0, :], dp[:, 1, :])
            det = s_pool.tile([P, F], F32, tag="det")
            nc.gpsimd.tensor_add(det[:], dta[:], dp[:, 2, :])
            adet = s_pool.tile([P, F], F32, tag="adet")
            nc.scalar.activation(adet[:], det[:], ACTF.Abs)
            # 1/|det|; exact-zero |det| (duplicate-node elements) -> NaN,
            # which must propagate into A and B to match the reference.
            recip = s_pool.tile([P, F], F32, tag="recip")
            nc.vector.reciprocal_approx_fast(out=recip[:], in_=adet[:])

            # --- G[i,d] = sum_n w_n[i] * r_n[d]   (fp16, broadcast APs)
            # accumulate the r1 term last: it has the longest dependency
            # chain (GPSIMD crosses -> ScalarE downconvert -> DVE)
            w2 = w_blk[:, 3:6, :].unsqueeze(2).broadcast_to([P, 3, 3, F])
            r2b = rh[:, 3:6, :].unsqueeze(1).broadcast_to([P, 3, 3, F])
            nc.vector.tensor_mul(G[:], w2, r2b)
            w3 = w_blk[:, 6:9, :].unsqueeze(2).broadcast_to([P, 3, 3, F])
            r3b = rh[:, 6:9, :].unsqueeze(1).broadcast_to([P, 3, 3, F])
            nc.vector.tensor_mul(gt[:], w3, r3b)
            nc.vector.tensor_add(G[:], G[:], gt[:])
            w1 = w_blk[:, 0:3, :].unsqueeze(2).broadcast_to([P, 3, 3, F])
            r1b = rh[:, 0:3, :].unsqueeze(1).broadcast_to([P, 3, 3, F])
            nc.vector.tensor_mul(gt[:], w1, r1b)
            nc.vector.tensor_add(G[:], G[:], gt[:])

            # --- S = sum_i G_ii^2 + 0.5 * sum_{i<j} (G_ij + G_ji)^2
            sqb = s_pool.tile([P, 6, F], F16, tag="sqb")
            for i in range(3):
                nc.scalar.activation(sqb[:, i, :], G[:, i, i, :], ACTF.Square)
            for k, (i, j) in enumerate(((0, 1), (0, 2), (1, 2))):
                nc.vector.tensor_add(sqb[:, 3 + k, :], G[:, i, j, :], G[:, j, i, :])
            nc.scalar.activation(sqb[:, 3:6, :], sqb[:, 3:6, :], ACTF.Square,
                                 scale=0.7071067811865476)
            sa = s_pool.tile([P, 3, F], F16, tag="sa")
            nc.vector.tensor_add(sa[:], sqb[:, 0:3, :], sqb[:, 3:6, :])
            s1 = s_pool.tile([P, F], F16, tag="s1")
            nc.vector.tensor_add(s1[:], sa[:, 0, :], sa[:, 1, :])
            S = s_pool.tile([P, F], F32, tag="S")
            nc.vector.tensor_add(S[:], s1[:], sa[:, 2, :])

            # --- T = tr(G);  T2 = T^2 (fp32 out)
            tt = s_pool.tile([P, F], F16, tag="tt")
            nc.vector.tensor_add(tt[:], G[:, 0, 0, :], G[:, 1, 1, :])
            T = s_pool.tile([P, F], F16, tag="T")
            nc.vector.tensor_add(T[:], tt[:], G[:, 2, 2, :])
            T2 = s_pool.tile([P, F], F32, tag="T2")
            nc.scalar.activation(T2[:], T[:], ACTF.Square)

            # --- partial sums (free-dim reduction on ScalarE via accum_out)
            junk = s_pool.tile([P, F], F16, tag="junk")
            scr = s_pool.tile([P, F], F32, tag="scr")
            nc.vector.tensor_mul(scr[:], S[:], recip[:])
            nc.scalar.activation(junk[:], scr[:], ACTF.Copy,
                                 accum_out=acc[:, 0, c:c + 1])
            scr2 = s_pool.tile([P, F], F32, tag="scr2")
            nc.vector.tensor_mul(scr2[:], T2[:], recip[:])
            nc.scalar.activation(junk[:], scr2[:], ACTF.Copy,
                                 accum_out=acc[:, 1, c:c + 1])
            scr3 = s_pool.tile([P, F], F32, tag="scr3")
            nc.vector.tensor_mul(scr3[:], uqz, adet[:])
            nc.scalar.activation(junk[:], scr3[:], ACTF.Copy,
                                 accum_out=acc[:, 2, c:c + 1])

        nc.sync.dma_start(out[:], acc[:])

    nc.compile()
    return nc


def _get_nc():
    if "nc" not in _CACHE:
        _CACHE["nc"] = _build()
    return _CACHE["nc"]


def _pack_core(conns_i, coords, us):
    """Gather + pack one core's elements into the two plane tensors."""
    xs = coords[conns_i]                     # [E,4,3]
    ue = us[conns_i]                         # [E,4,3]
    e = xs[:, 1:4, :] - xs[:, 0:1, :]        # [E,3,3]  (n, comp)
    w = ue[:, 1:4, :] - ue[:, 0:1, :]        # [E,3,3]
    uqzsum = ue[:, :, 2].sum(axis=1)         # [E]
    E = conns_i.shape[0]
    pad_e = np.array([1, 0, 0, 0, 1, 0, 0, 0, 1], dtype=np.float32)

    p32 = np.empty((N_P32, E_PAD), dtype=np.float32)
    p32[0:9, :E] = e.reshape(E, 9).T
    p32[9, :E] = uqzsum
    p32[0:9, E:] = pad_e[:, None]
    p32[9, E:] = 0.0

    p16 = np.empty((N_P16, E_PAD), dtype=np.float16)
    eh = np.empty((3, 3, E_PAD), dtype=np.float16)          # [vec n, comp, elem]
    eh[:, :, :E] = e.reshape(E, 9).T.astype(np.float16).reshape(3, 3, E)
    eh[:, :, E:] = pad_e.astype(np.float16).reshape(3, 3)[:, :, None]
    rot1 = eh[:, [1, 2, 0], :]                              # (vy, vz, vx)
    rot2 = eh[:, [2, 0, 1], :]                              # (vz, vx, vy)
    # A = [rot1(e3), rot1(e1)]; B = [rot2(e1), rot2(e2)]
    # C = [rot2(e3), rot2(e1)]; D = [rot1(e1), rot1(e2)]
    p16[0:3] = rot1[2]; p16[3:6] = rot1[0]
    p16[6:9] = rot2[0]; p16[9:12] = rot2[1]
    p16[12:15] = rot2[2]; p16[15:18] = rot2[0]
    p16[18:21] = rot1[0]; p16[21:24] = rot1[1]
    p16[24:33, :E] = w.reshape(E, 9).T.astype(np.float16)
    p16[24:33, E:] = 0.0

    return (np.ascontiguousarray(p32.reshape(N_P32, P, COLS).transpose(1, 0, 2)),
            np.ascontiguousarray(p16.reshape(N_P16, P, COLS).transpose(1, 0, 2)))


def kernel(params, coords, us, t, conns):
    params = np.asarray(params, np.float32)
    coords = np.asarray(coords, np.float32)
    us = np.asarray(us, np.float32)
    conns = np.asarray(conns)
    lam, mu, rho = (np.float64(params[0]), np.float64(params[1]),
                    np.float64(params[2]))

    nc = _get_nc()
    in_maps = []
    for i in range(N_CORES):
        ci = conns[i * E_CORE:(i + 1) * E_CORE]
        a32, a16 = _pack_core(ci, coords, us)
        in_maps.append({"planes32": a32, "planes16": a16})

    res = run_bass_kernel_spmd(nc, in_maps, core_ids=list(range(N_CORES)),
                               trace=bool(_CACHE.get("trace", False)))
    _CACHE["last_results"] = res

    total = np.float64(0.0)
    for i in range(N_CORES):
        o = res.results[i]["out"].astype(np.float64)   # [P, 3, N_CHUNKS]
        A = o[:, 0, :].sum()
        B = o[:, 1, :].sum()
        C = o[:, 2, :].sum()
        total += mu / 6.0 * A + lam / 12.0 * B - rho / 24.0 * C
    return np.asarray(total, dtype=np.float32)
